# revision 26
# baseline (speedup 1.0000x reference)
"""Trainium2 Bass kernel for batched masked attention (Z=8, S=2048, D=1024).

Strategy: pure data-parallel over batch z — each of the 8 NeuronCores computes
full attention for one batch element. No collectives.

Mask compaction: the reference's symmetric mask kills row q and column k
whenever position is masked (masked-query rows are exactly 0 in the output,
masked-key columns contribute exactly 0 to every sum). Query-mask == key-mask,
so the host gathers only the unmasked positions (~half), padded to a multiple
of 64 shared across cores, runs dense attention on the compacted sequence,
and scatters the result rows back into a zero output. Bit-equivalent math at
~40% of the dense FLOPs.

Score-projection fusion: z = (x Wq^T)(x Wk^T)^T = x (Wq^T Wk) x^T, so the
host precomputes G = Wq^T @ Wk once (f32) and the kernel runs a single score
projection m = x @ G instead of separate q and k projections — the z matmul
then contracts the already-resident xcT tiles against mT. One fewer GEMM on
the PE and 2MB less input DMA.

Per-core dataflow (all matmuls, no on-chip transposes):
  - host passes xcT = x[z][idx].T  [D, N] (bf16), G = Wq^T Wk (bf16),
    Wv.T (bf16), bv (f32)
  - mT[j,s]           = G-tile.T @ xcT         (PE)
  - v[s,a]            = xcT-tile.T @ Wv.T + bv (bias added on DVE from a
                                                partition-broadcast bv row)
  - zT[k,q]           = xcT-tile.T @ mT        (scores with keys on partitions)
  - ET                = exp(zT/32 + kbias[k])  (ScalarE; padding keys get
                                                bias -30000 -> exp underflows to 0)
  - out_psum[q,a]     = ET-tile.T @ v          (PE, contraction over keys)
  - denom[q]          = ET-tile.T @ ones       (same stationary, N=1 matmul)
  - out[q,a]          = out_psum / denom[q]    (DVE, per-partition scale)

No softmax max-subtraction is needed (logits ~ N(0,1); exp is safe in f32),
which is what lets the division defer to the output and keeps every stage in
a matmul-friendly layout. PE is pre-warmed with dummy matmuls during the input
DMA lead-in; the final chunk's output stores are split across partition
stripes so the tail doesn't wait on a single DMA queue.
"""

import numpy as np
import ml_dtypes

P = 128
S = 2048  # full sequence length
D = 1024  # model dim (= dim_qk = dim_v)
NI = D // P  # 8 contraction tiles for projections
VC = 512  # v free-dim chunk
NVC = D // VC  # 2
SCALE = 1.0 / 32.0  # 1/sqrt(D)
GRAN = 64  # sequence padding granularity
NWARM = 40  # PE pre-warm dummy matmuls
OSPLIT = 16  # output-store partition split (final chunk only)

_CACHE = {}


def _chunks(total, maxw):
    out = []
    off = 0
    while off < total:
        w = min(maxw, total - off)
        out.append((off, w))
        off += w
    return out


def _build_nc(N):
    """Build the per-core graph for a compacted, padded sequence length N."""
    from contextlib import ExitStack

    import concourse.tile as tile
    from concourse import bacc, mybir
    from concourse.bass import ts, ds

    f32 = mybir.dt.float32
    bf16 = mybir.dt.bfloat16
    EXP = mybir.ActivationFunctionType.Exp

    ktiles = _chunks(N, P)  # [(koff, kh)] kh in {128, 64}
    nkt = len(ktiles)
    qchunks = _chunks(N, 512)

    nc = bacc.Bacc(None, target_bir_lowering=False, debug=False)

    xc_d = nc.declare_dram_parameter("xc", [D, N], bf16, isOutput=False)
    wm_d = nc.declare_dram_parameter("wm", [D, D], bf16, isOutput=False)
    wv_d = nc.declare_dram_parameter("wv", [D, D], bf16, isOutput=False)
    bv_d = nc.declare_dram_parameter("bv", [1, D], f32, isOutput=False)
    kb_d = nc.declare_dram_parameter("kbias", [P, nkt], f32, isOutput=False)
    out_d = nc.declare_dram_parameter("out", [N, D], bf16, isOutput=True)

    with tile.TileContext(nc) as tc, ExitStack() as st:
        const = st.enter_context(tc.tile_pool(name="const", bufs=1))
        persist = st.enter_context(tc.tile_pool(name="persist", bufs=1))
        # one PSUM ring shared by every stage — no pool-boundary barriers
        ps = st.enter_context(tc.tile_pool(name="ps", bufs=8, space="PSUM"))

        def psum(name, h, w):
            t = ps.tile([P, 512], f32, name=name, tag="ps")
            return t[:h, :w]

        ones_col = const.tile([P, 1], bf16, name="ones_col", tag="ones_col")
        nc.gpsimd.memset(ones_col, 1.0)

        # PE pre-warm: dummy matmuls with no data deps run during the input
        # DMA lead-in so HAM un-throttles before the first real matmul.
        ws = const.tile([P, P], bf16, name="ws", tag="ws")
        nc.gpsimd.memset(ws, 0.0)
        for i in range(NWARM):
            wp = psum(f"wp{i}", P, P)
            nc.tensor.matmul(wp, lhsT=ws, rhs=ws, start=True, stop=True)

        # xc stays resident through phase 2 (the z matmul contracts it)
        xts, wm_t = [], []
        mt = [
            persist.tile([P, N], bf16, name=f"mt{a}", tag="mt", bufs=NI)
            for a in range(NI)
        ]
        v = [
            persist.tile([P, D], bf16, name=f"v{s}", tag="v", bufs=nkt)
            for s in range(nkt)
        ]

        # ---- phase 1: projections -------------------------------------
        with tc.tile_pool(name="xw", bufs=1) as xw:
            # interleave xc / wm loads so the m-projection is fed first; the
            # first pair is split in half so it spreads over more DMA queues
            # and the first real matmul starts sooner
            for it in range(NI):
                t = persist.tile([P, N], bf16, name=f"xtile{it}", tag="xt", bufs=NI)
                if it == 0:
                    nc.sync.dma_start(t[:64], xc_d[: P // 2, :])
                    nc.sync.dma_start(t[64:], xc_d[P // 2 : P, :])
                else:
                    nc.sync.dma_start(t, xc_d[ts(it, P), :])
                xts.append(t)
                w = xw.tile([P, D], bf16, name=f"wmt{it}", tag="w", bufs=16)
                if it == 0:
                    nc.sync.dma_start(w[:64], wm_d[: P // 2, :])
                    nc.sync.dma_start(w[64:], wm_d[P // 2 : P, :])
                else:
                    nc.sync.dma_start(w, wm_d[ts(it, P), :])
                wm_t.append(w)

            kb_sb = const.tile([P, nkt], f32, name="kb_sb", tag="kb_sb")
            nc.scalar.dma_start(kb_sb, kb_d[:, :])
            bv_sb = const.tile([1, D], f32, name="bv_sb", tag="bv_sb")
            nc.scalar.dma_start(bv_sb, bv_d[:, :])
            bv_bc = const.tile([P, D], f32, name="bv_bc", tag="bv_bc")
            nc.gpsimd.partition_broadcast(bv_bc, bv_sb[:1, :])

            wv_t = []
            for it in range(NI):
                w = xw.tile([P, D], bf16, name=f"wvt{it}", tag="w", bufs=16)
                nc.sync.dma_start(w, wv_d[ts(it, P), :])
                wv_t.append(w)

            # mT: out[j-tile, chunk] = sum_i G[i, j-tile].T @ xcT[i, chunk]
            for a in range(NI):
                pss = [
                    psum(f"pp_m{a}_{ci}", P, w) for ci, (off, w) in enumerate(qchunks)
                ]
                for it in range(NI):
                    for ci, (off, w) in enumerate(qchunks):
                        nc.tensor.matmul(
                            pss[ci],
                            lhsT=wm_t[it][:, ts(a, P)],
                            rhs=xts[it][:, ds(off, w)],
                            start=(it == 0),
                            stop=(it == NI - 1),
                        )
                for ci, (off, w) in enumerate(qchunks):
                    nc.vector.tensor_copy(mt[a][:, ds(off, w)], pss[ci])

            # v: out[k-tile, chunk] = sum_i xcT[i, k-tile].T @ Wv.T[i, chunk] + bv
            for s16, (koff, kh) in enumerate(ktiles):
                pss = [psum(f"pp_v{s16}_{c}", kh, VC) for c in range(NVC)]
                for it in range(NI):
                    for c in range(NVC):
                        nc.tensor.matmul(
                            pss[c],
                            lhsT=xts[it][:, ds(koff, kh)],
                            rhs=wv_t[it][:, ts(c, VC)],
                            start=(it == 0),
                            stop=(it == NI - 1),
                        )
                for c in range(NVC):
                    nc.vector.tensor_add(
                        v[s16][:kh, ts(c, VC)], pss[c], bv_bc[:kh, ts(c, VC)]
                    )

        # ---- phase 2: attention ---------------------------------------
        with (
            tc.tile_pool(name="etp", bufs=1) as etp,
            tc.tile_pool(name="outp", bufs=4) as outp,
            tc.tile_pool(name="smol", bufs=8) as smol,
        ):
            for qc, (qoff, qw) in enumerate(qchunks):
                ets = []
                for k16, (koff, kh) in enumerate(ktiles):
                    zps = psum(f"z{qc}_{k16}", kh, qw)
                    for a in range(NI):
                        nc.tensor.matmul(
                            zps,
                            lhsT=xts[a][:, ds(koff, kh)],
                            rhs=mt[a][:, ds(qoff, qw)],
                            start=(a == 0),
                            stop=(a == NI - 1),
                        )
                    et = etp.tile(
                        [P, 512],
                        bf16,
                        name=f"et{qc}_{k16}",
                        tag="et",
                        bufs=2 * nkt,
                    )[:kh, :qw]
                    nc.scalar.activation(
                        et, zps, EXP, bias=kb_sb[:kh, k16 : k16 + 1], scale=SCALE
                    )
                    ets.append(et)

                for qsoff, qh in _chunks(qw, P):
                    qrow = qoff + qsoff  # global compacted row
                    opss = [psum(f"pv{qrow}_{c}", qh, VC) for c in range(NVC)]
                    dps = psum(f"dn{qrow}", qh, 1)
                    for k16, (koff, kh) in enumerate(ktiles):
                        lhs = ets[k16][:, ds(qsoff, qh)]
                        for c in range(NVC):
                            nc.tensor.matmul(
                                opss[c],
                                lhsT=lhs,
                                rhs=v[k16][:kh, ts(c, VC)],
                                start=(k16 == 0),
                                stop=(k16 == nkt - 1),
                            )
                        nc.tensor.matmul(
                            dps,
                            lhsT=lhs,
                            rhs=ones_col[:kh, :1],
                            start=(k16 == 0),
                            stop=(k16 == nkt - 1),
                        )
                    rec = smol.tile([P, 1], f32, name=f"rec{qrow}", tag="rec")[:qh]
                    nc.vector.reciprocal(rec, dps)
                    for c in range(NVC):
                        ot = outp.tile([P, VC], bf16, name=f"ot{qrow}_{c}", tag="ot")[
                            :qh
                        ]
                        nc.vector.tensor_scalar_mul(ot, opss[c], rec)
                        # alternate the issuing sequencer so the two stores of
                        # a q-sub trigger in parallel (each lands on its own
                        # DMA queue; splitting further only adds issue cost)
                        eng = nc.sync if c == 0 else nc.scalar
                        eng.dma_start(out_d[ds(qrow, qh), ts(c, VC)], ot)

    nc.compile()
    return nc


def _get_nc(N):
    if N not in _CACHE:
        _CACHE[N] = _build_nc(N)
    return _CACHE[N]


def _make_in_maps(x, Wq, Wk, Wv, bv, mask, idxs, N):
    bf16 = ml_dtypes.bfloat16
    ktiles = _chunks(N, P)
    nkt = len(ktiles)
    G = np.float32(Wq).T @ np.float32(Wk)  # z = x G x^T
    wm = np.ascontiguousarray(G).astype(bf16)
    wv_t = np.ascontiguousarray(Wv.astype(np.float32).T).astype(bf16)
    bv_row = np.ascontiguousarray(bv.astype(np.float32).reshape(1, D))
    in_maps = []
    for z in range(8):
        idx = idxs[z]
        n = idx.size
        idx_pad = np.zeros(N, dtype=np.int64)
        idx_pad[:n] = idx
        xc = np.ascontiguousarray(x[z][idx_pad].astype(np.float32).T).astype(bf16)
        kb = np.full(N, -30000.0, dtype=np.float32)
        kb[:n] = 0.0
        # kbias SBUF layout: column j covers compacted rows koff_j..koff_j+kh_j
        kbm = np.full((P, nkt), -30000.0, dtype=np.float32)
        for j, (koff, kh) in enumerate(ktiles):
            kbm[:kh, j] = kb[koff : koff + kh]
        in_maps.append(
            {
                "xc": xc,
                "wm": wm,
                "wv": wv_t,
                "bv": bv_row,
                "kbias": np.ascontiguousarray(kbm),
            }
        )
    return in_maps


def run(x, Wq, Wk, Wv, bv, mask, trace=False):
    from concourse.bass_utils import run_bass_kernel_spmd

    x = np.asarray(x)
    mask = np.asarray(mask).astype(bool)
    idxs = [np.nonzero(~mask[z])[0] for z in range(8)]
    nmax = max(int(i.size) for i in idxs)
    N = max(GRAN, -(-nmax // GRAN) * GRAN)  # shared padded length
    nc = _get_nc(N)
    in_maps = _make_in_maps(x, Wq, Wk, Wv, bv, mask, idxs, N)
    res = run_bass_kernel_spmd(nc, in_maps, core_ids=list(range(8)), trace=trace)
    out = np.zeros((8, S, D), dtype=np.float32)
    for z in range(8):
        n = idxs[z].size
        if n:
            out[z][idxs[z]] = res.results[z]["out"][:n].astype(np.float32)
    return out, res


def kernel(x, Wq, Wk, Wv, bv, mask):
    out, _ = run(x, Wq, Wk, Wv, bv, mask, trace=False)
    return out


# revision 27
# speedup vs baseline: 1.0050x; 1.0050x over previous
"""Trainium2 Bass kernel for batched masked attention (Z=8, S=2048, D=1024).

Strategy: pure data-parallel over batch z — each of the 8 NeuronCores computes
full attention for one batch element. No collectives.

Mask compaction: the reference's symmetric mask kills row q and column k
whenever position is masked (masked-query rows are exactly 0 in the output,
masked-key columns contribute exactly 0 to every sum). Query-mask == key-mask,
so the host gathers only the unmasked positions (~half), padded to a multiple
of 64 shared across cores, runs dense attention on the compacted sequence,
and scatters the result rows back into a zero output. Bit-equivalent math at
~40% of the dense FLOPs.

Score-projection fusion: z = (x Wq^T)(x Wk^T)^T = x (Wq^T Wk) x^T, so the
host precomputes G = Wq^T @ Wk once (f32) and the kernel runs a single score
projection m = x @ G instead of separate q and k projections — the z matmul
then contracts the already-resident xcT tiles against mT. One fewer GEMM on
the PE and 2MB less input DMA.

Per-core dataflow (all matmuls, no on-chip transposes):
  - host passes xcT = x[z][idx].T  [D, N] (bf16), G = Wq^T Wk (bf16),
    Wv.T (bf16), bv (f32)
  - mT[j,s]           = G-tile.T @ xcT         (PE)
  - v[s,a]            = xcT-tile.T @ Wv.T + bv (bias added on DVE from a
                                                partition-broadcast bv row)
  - zT[k,q]           = xcT-tile.T @ mT        (scores with keys on partitions)
  - ET                = exp(zT/32 + kbias[k])  (ScalarE; padding keys get
                                                bias -30000 -> exp underflows to 0)
  - out_psum[q,a]     = ET-tile.T @ v          (PE, contraction over keys)
  - denom[q]          = ET-tile.T @ ones       (same stationary, N=1 matmul)
  - out[q,a]          = out_psum / denom[q]    (DVE, per-partition scale)

No softmax max-subtraction is needed (logits ~ N(0,1); exp is safe in f32),
which is what lets the division defer to the output and keeps every stage in
a matmul-friendly layout. PE is pre-warmed with dummy matmuls during the input
DMA lead-in; the final chunk's output stores are split across partition
stripes so the tail doesn't wait on a single DMA queue.
"""

import numpy as np
import ml_dtypes

P = 128
S = 2048  # full sequence length
D = 1024  # model dim (= dim_qk = dim_v)
NI = D // P  # 8 contraction tiles for projections
VC = 512  # v free-dim chunk
NVC = D // VC  # 2
SCALE = 1.0 / 32.0  # 1/sqrt(D)
GRAN = 64  # sequence padding granularity
NWARM = 40  # PE pre-warm dummy matmuls
OSPLIT = 16  # output-store partition split (final chunk only)

_CACHE = {}


def _chunks(total, maxw):
    out = []
    off = 0
    while off < total:
        w = min(maxw, total - off)
        out.append((off, w))
        off += w
    return out


def _build_nc(N):
    """Build the per-core graph for a compacted, padded sequence length N."""
    from contextlib import ExitStack

    import concourse.tile as tile
    from concourse import bacc, mybir
    from concourse.bass import ts, ds

    f32 = mybir.dt.float32
    bf16 = mybir.dt.bfloat16
    EXP = mybir.ActivationFunctionType.Exp

    ktiles = _chunks(N, P)  # [(koff, kh)] kh in {128, 64}
    nkt = len(ktiles)
    qchunks = _chunks(N, 512)

    nc = bacc.Bacc(None, target_bir_lowering=False, debug=False)

    xc_d = nc.declare_dram_parameter("xc", [D, N], bf16, isOutput=False)
    wm_d = nc.declare_dram_parameter("wm", [D, D], bf16, isOutput=False)
    wv_d = nc.declare_dram_parameter("wv", [D, D], bf16, isOutput=False)
    bv_d = nc.declare_dram_parameter("bv", [1, D], f32, isOutput=False)
    kb_d = nc.declare_dram_parameter("kbias", [P, nkt], f32, isOutput=False)
    out_d = nc.declare_dram_parameter("out", [N, D], f32, isOutput=True)

    with tile.TileContext(nc) as tc, ExitStack() as st:
        const = st.enter_context(tc.tile_pool(name="const", bufs=1))
        persist = st.enter_context(tc.tile_pool(name="persist", bufs=1))
        # one PSUM ring shared by every stage — no pool-boundary barriers
        ps = st.enter_context(tc.tile_pool(name="ps", bufs=8, space="PSUM"))

        def psum(name, h, w):
            t = ps.tile([P, 512], f32, name=name, tag="ps")
            return t[:h, :w]

        ones_col = const.tile([P, 1], bf16, name="ones_col", tag="ones_col")
        nc.gpsimd.memset(ones_col, 1.0)

        # PE pre-warm: dummy matmuls with no data deps run during the input
        # DMA lead-in so HAM un-throttles before the first real matmul.
        ws = const.tile([P, P], bf16, name="ws", tag="ws")
        nc.gpsimd.memset(ws, 0.0)
        for i in range(NWARM):
            wp = psum(f"wp{i}", P, P)
            nc.tensor.matmul(wp, lhsT=ws, rhs=ws, start=True, stop=True)

        # xc stays resident through phase 2 (the z matmul contracts it)
        xts, wm_t = [], []
        mt = [
            persist.tile([P, N], bf16, name=f"mt{a}", tag="mt", bufs=NI)
            for a in range(NI)
        ]
        v = [
            persist.tile([P, D], bf16, name=f"v{s}", tag="v", bufs=nkt)
            for s in range(nkt)
        ]

        # ---- phase 1: projections -------------------------------------
        with tc.tile_pool(name="xw", bufs=1) as xw:
            # interleave xc / wm loads so the m-projection is fed first; the
            # first pair is split in half so it spreads over more DMA queues
            # and the first real matmul starts sooner
            for it in range(NI):
                t = persist.tile([P, N], bf16, name=f"xtile{it}", tag="xt", bufs=NI)
                if it == 0:
                    nc.sync.dma_start(t[:64], xc_d[: P // 2, :])
                    nc.sync.dma_start(t[64:], xc_d[P // 2 : P, :])
                else:
                    nc.sync.dma_start(t, xc_d[ts(it, P), :])
                xts.append(t)
                w = xw.tile([P, D], bf16, name=f"wmt{it}", tag="w", bufs=16)
                if it == 0:
                    nc.sync.dma_start(w[:64], wm_d[: P // 2, :])
                    nc.sync.dma_start(w[64:], wm_d[P // 2 : P, :])
                else:
                    nc.sync.dma_start(w, wm_d[ts(it, P), :])
                wm_t.append(w)

            kb_sb = const.tile([P, nkt], f32, name="kb_sb", tag="kb_sb")
            nc.scalar.dma_start(kb_sb, kb_d[:, :])
            bv_sb = const.tile([1, D], f32, name="bv_sb", tag="bv_sb")
            nc.scalar.dma_start(bv_sb, bv_d[:, :])
            bv_bc = const.tile([P, D], f32, name="bv_bc", tag="bv_bc")
            nc.gpsimd.partition_broadcast(bv_bc, bv_sb[:1, :])

            wv_t = []
            for it in range(NI):
                w = xw.tile([P, D], bf16, name=f"wvt{it}", tag="w", bufs=16)
                nc.sync.dma_start(w, wv_d[ts(it, P), :])
                wv_t.append(w)

            # mT: out[j-tile, chunk] = sum_i G[i, j-tile].T @ xcT[i, chunk]
            for a in range(NI):
                pss = [
                    psum(f"pp_m{a}_{ci}", P, w) for ci, (off, w) in enumerate(qchunks)
                ]
                for it in range(NI):
                    for ci, (off, w) in enumerate(qchunks):
                        nc.tensor.matmul(
                            pss[ci],
                            lhsT=wm_t[it][:, ts(a, P)],
                            rhs=xts[it][:, ds(off, w)],
                            start=(it == 0),
                            stop=(it == NI - 1),
                        )
                for ci, (off, w) in enumerate(qchunks):
                    nc.vector.tensor_copy(mt[a][:, ds(off, w)], pss[ci])

            # v: out[k-tile, chunk] = sum_i xcT[i, k-tile].T @ Wv.T[i, chunk] + bv
            for s16, (koff, kh) in enumerate(ktiles):
                pss = [psum(f"pp_v{s16}_{c}", kh, VC) for c in range(NVC)]
                for it in range(NI):
                    for c in range(NVC):
                        nc.tensor.matmul(
                            pss[c],
                            lhsT=xts[it][:, ds(koff, kh)],
                            rhs=wv_t[it][:, ts(c, VC)],
                            start=(it == 0),
                            stop=(it == NI - 1),
                        )
                for c in range(NVC):
                    nc.vector.tensor_add(
                        v[s16][:kh, ts(c, VC)], pss[c], bv_bc[:kh, ts(c, VC)]
                    )

        # ---- phase 2: attention ---------------------------------------
        with (
            tc.tile_pool(name="etp", bufs=1) as etp,
            tc.tile_pool(name="outp", bufs=4) as outp,
            tc.tile_pool(name="smol", bufs=8) as smol,
        ):
            for qc, (qoff, qw) in enumerate(qchunks):
                ets = []
                for k16, (koff, kh) in enumerate(ktiles):
                    zps = psum(f"z{qc}_{k16}", kh, qw)
                    for a in range(NI):
                        nc.tensor.matmul(
                            zps,
                            lhsT=xts[a][:, ds(koff, kh)],
                            rhs=mt[a][:, ds(qoff, qw)],
                            start=(a == 0),
                            stop=(a == NI - 1),
                        )
                    et = etp.tile(
                        [P, 512],
                        bf16,
                        name=f"et{qc}_{k16}",
                        tag="et",
                        bufs=2 * nkt,
                    )[:kh, :qw]
                    nc.scalar.activation(
                        et, zps, EXP, bias=kb_sb[:kh, k16 : k16 + 1], scale=SCALE
                    )
                    ets.append(et)

                for qsoff, qh in _chunks(qw, P):
                    qrow = qoff + qsoff  # global compacted row
                    opss = [psum(f"pv{qrow}_{c}", qh, VC) for c in range(NVC)]
                    dps = psum(f"dn{qrow}", qh, 1)
                    for k16, (koff, kh) in enumerate(ktiles):
                        lhs = ets[k16][:, ds(qsoff, qh)]
                        for c in range(NVC):
                            nc.tensor.matmul(
                                opss[c],
                                lhsT=lhs,
                                rhs=v[k16][:kh, ts(c, VC)],
                                start=(k16 == 0),
                                stop=(k16 == nkt - 1),
                            )
                        nc.tensor.matmul(
                            dps,
                            lhsT=lhs,
                            rhs=ones_col[:kh, :1],
                            start=(k16 == 0),
                            stop=(k16 == nkt - 1),
                        )
                    rec = smol.tile([P, 1], f32, name=f"rec{qrow}", tag="rec")[:qh]
                    nc.vector.reciprocal(rec, dps)
                    for c in range(NVC):
                        ot = outp.tile([P, VC], f32, name=f"ot{qrow}_{c}", tag="ot")[
                            :qh
                        ]
                        nc.vector.tensor_scalar_mul(ot, opss[c], rec)
                        # alternate the issuing sequencer so the two stores of
                        # a q-sub trigger in parallel (each lands on its own
                        # DMA queue; splitting further only adds issue cost)
                        eng = nc.sync if c == 0 else nc.scalar
                        eng.dma_start(out_d[ds(qrow, qh), ts(c, VC)], ot)

    nc.compile()
    return nc


def _get_nc(N):
    if N not in _CACHE:
        _CACHE[N] = _build_nc(N)
    return _CACHE[N]


def _make_in_maps(x, Wq, Wk, Wv, bv, mask, idxs, N):
    bf16 = ml_dtypes.bfloat16
    ktiles = _chunks(N, P)
    nkt = len(ktiles)
    G = np.float32(Wq).T @ np.float32(Wk)  # z = x G x^T
    wm = np.ascontiguousarray(G).astype(bf16)
    wv_t = np.ascontiguousarray(Wv.astype(np.float32).T).astype(bf16)
    bv_row = np.ascontiguousarray(bv.astype(np.float32).reshape(1, D))
    in_maps = []
    for z in range(8):
        idx = idxs[z]
        n = idx.size
        idx_pad = np.zeros(N, dtype=np.int64)
        idx_pad[:n] = idx
        xc = np.ascontiguousarray(x[z][idx_pad].astype(np.float32).T).astype(bf16)
        kb = np.full(N, -30000.0, dtype=np.float32)
        kb[:n] = 0.0
        # kbias SBUF layout: column j covers compacted rows koff_j..koff_j+kh_j
        kbm = np.full((P, nkt), -30000.0, dtype=np.float32)
        for j, (koff, kh) in enumerate(ktiles):
            kbm[:kh, j] = kb[koff : koff + kh]
        in_maps.append(
            {
                "xc": xc,
                "wm": wm,
                "wv": wv_t,
                "bv": bv_row,
                "kbias": np.ascontiguousarray(kbm),
            }
        )
    return in_maps


def run(x, Wq, Wk, Wv, bv, mask, trace=False):
    from concourse.bass_utils import run_bass_kernel_spmd

    x = np.asarray(x)
    mask = np.asarray(mask).astype(bool)
    idxs = [np.nonzero(~mask[z])[0] for z in range(8)]
    nmax = max(int(i.size) for i in idxs)
    N = max(GRAN, -(-nmax // GRAN) * GRAN)  # shared padded length
    nc = _get_nc(N)
    in_maps = _make_in_maps(x, Wq, Wk, Wv, bv, mask, idxs, N)
    res = run_bass_kernel_spmd(nc, in_maps, core_ids=list(range(8)), trace=trace)
    out = np.zeros((8, S, D), dtype=np.float32)
    for z in range(8):
        n = idxs[z].size
        if n:
            out[z][idxs[z]] = res.results[z]["out"][:n].astype(np.float32)
    return out, res


def kernel(x, Wq, Wk, Wv, bv, mask):
    out, _ = run(x, Wq, Wk, Wv, bv, mask, trace=False)
    return out
